# revision 69
# baseline (speedup 1.0000x reference)
"""EdgeCrossingsLoss Trainium2 kernel (8-core SPMD, data-parallel over query faces).

Two device launches (no usable on-device gather in this runtime; the host does
the small index-merge + geometry gather between the launches):

prog1 (per core, 1280 query rows = 10 tiles of 128):
  PE:  -d2[q, c] for all 10240 candidates via a K=16 bf16 hi/lo-split matmul
       (bf16 products are exact, accumulated in f32 PSUM). rhs sits in four
       16-partition bands at base partitions 0/32/64/96. PSUM pieces are
       [128, 1024] f32 (2 banks), a 4-deep ring.
  Reduction: instead of two full f32 DVE scans (max8 + max_index, the old
       bottleneck), window-4 maxima of -d2 are computed in bf16 and shipped
       whole. TRN2 limits: GPSIMD cannot touch PSUM (and lacks max/compare
       ops in this runtime); a TensorTensor reads at most one PSUM operand.
       So 4 pieces/tile drain via DVE tensor_reduce (single strided PSUM
       input, reduce AND window in one op), 6 drain via ACT copy to bf16
       SBUF followed by a 2-level contiguous-half pair-max tree on DVE (bf16
       2x mode; one level-2 per tile on GPSIMD via max(x,y)=y+relu(x-y)).
       The [128, 2560] window-max tile is DMA'd out; no on-device top-k.

host: picks the top-32 windows per row from the 2560 bf16 window maxima
      (window W holds candidates 1024*(W//256) + W%256 + 256*t, t=0..3),
      resolves the 32*4 member candidates exactly (f32), takes the exact
      top-16 with the jax tie-break. Rows where the 33rd-best window max
      could hide a true top-16 member (value margin covering bf16 rounding)
      are recomputed exactly (vectorized, ~a few % of rows). Gathers the 16
      neighbor faces' edge geometry; folds probabilities and the
      self-neighbor mask into per-(row, slot) weights.

prog2 (per core): 3x3 line-line crossing tests over DEDUPLICATED face pairs.
      The 3x3-summed hit count is symmetric (hit(i,j) == hit(j,i)), so the
      host sends each unique pair once, packed densely across cores (~65% of
      the raw 10000x16 pair slots), and assembles per-row crossing counts
      from the shared results. Engine-split across four software-pipelined
      slot-chunks: DVE runs the broadcast-AP ops, the compare and the
      reduction + part of the unit-stride chain, GPSIMD runs adds/subs/mults,
      ACT squares cr^2 and (num/eps)^2 (eps folded into the square's scale).
      hit = (num/eps)^2 < |cross|^2 (den=0 / NaN / zero-padded slots fall out
      correctly as no-hit).

Host computes loss = dot(p_i over valid (i,j) slots, hit_unique[pair]) / F.
"""
import os
import numpy as np
import ml_dtypes
from contextlib import ExitStack

import concourse.bass as bass
import concourse.tile as tile
import concourse.bacc as bacc
from concourse import mybir
from concourse.bass_utils import run_bass_kernel_spmd

F32 = mybir.dt.float32
BF16 = mybir.dt.bfloat16
U16 = mybir.dt.uint16

NCORES = 8
KNN = 16
EPS = 1e-5
FP = 10240            # padded candidate count
NR = FP // NCORES     # 1280 rows per core
NT = NR // 128        # 10 tiles of 128 rows
KMM = 16              # matmul contraction rows (bf16 hi/lo split)
NGRP = 4              # rhs partition bands (at partitions 0/32/64/96)
GW = FP // NGRP       # 2560 candidates per band
PIECE = 1024          # PSUM piece width (f32, exactly 2 banks -> 4-deep ring)
NPIECE = FP // PIECE  # 10 pieces per tile
MMCH = 512            # matmul N per instruction (one PSUM bank)
NWIN = FP // 4        # 2560 window-4 maxima per row
MWIN = 32             # host resolves top-32 windows per row
KINDS = "DADAADADAA"  # prog1 piece drain engines: A=ACT copy, D=DVE reduce
# wm slot -> piece map (batched tree instrs need contiguous wm slots):
# slots 0-2 = DVE level-2 batch (A-pieces 6,8,9), 3-5 = GPSIMD level-2
# (A-pieces 1,3,4), 6-9 = DVE reduces (D-pieces 0,2,5,7)
WM_PERM = np.array([6, 8, 9, 1, 3, 4, 0, 2, 5, 7], dtype=np.int64)

ALU = mybir.AluOpType
AFT = mybir.ActivationFunctionType


def _build_prog1():
    nc = bacc.Bacc("TRN2", target_bir_lowering=False, debug=False,
                   num_devices=NCORES)
    # band b occupies partitions [32b, 32b+16); lhsT replicated into each band
    lhsT_in = nc.dram_tensor("lhsT", [128, NR], BF16, kind="ExternalInput").ap()
    rhs_in = nc.dram_tensor("rhs", [128, GW], BF16, kind="ExternalInput").ap()
    wm_out = nc.dram_tensor("wm", [NT, 128, NWIN], BF16,
                            kind="ExternalOutput").ap()

    with tile.TileContext(nc) as tc, ExitStack() as ctx:
        const_pool = ctx.enter_context(tc.tile_pool(name="const", bufs=1))
        psum_pool = ctx.enter_context(tc.tile_pool(name="psum", bufs=4,
                                                   space="PSUM"))
        ab_pool = ctx.enter_context(tc.tile_pool(name="ab", bufs=3))
        l1_pool = ctx.enter_context(tc.tile_pool(name="l1", bufs=6))
        wm_pool = ctx.enter_context(tc.tile_pool(name="wmp", bufs=3))

        # tile 0's lhsT slice and the first rhs chunk land first so the first
        # matmul issues ~1us earlier (the HWDGE serializes DMA descriptors)
        lhsT_sb = const_pool.tile([128, NR], BF16)
        nc.sync.dma_start(lhsT_sb[:, 0:128], lhsT_in[:, 0:128])
        rhs_sb = const_pool.tile([128, GW], BF16)
        nc.scalar.dma_start(rhs_sb[:, 0:GW // 4], rhs_in[:, 0:GW // 4])
        nc.sync.dma_start(lhsT_sb[:, 128:], lhsT_in[:, 128:])
        for j in range(1, 4):
            eng = (nc.scalar, nc.sync)[j % 2]
            eng.dma_start(rhs_sb[:, j * (GW // 4):(j + 1) * (GW // 4)],
                          rhs_in[:, j * (GW // 4):(j + 1) * (GW // 4)])

        # PSUM-read rules on TRN2: GPSIMD may not touch PSUM at all, and a
        # TensorTensor may read at most ONE operand from PSUM. GPSIMD also
        # lacks max/min/compare ops in this runtime (add/sub/mult and
        # tensor_scalar max only). So: DVE tensor_reduce (single PSUM input,
        # strided [128, 256, 4] view) drains AND window-4-maxes 4 pieces per
        # tile in one op each; ACT copies the other 6 pieces to bf16 SBUF
        # where the 2-level contiguous-half pair-max tree runs on DVE (bf16
        # 2x) or on GPSIMD via the 3-op identity max(x,y) = y + relu(x-y).
        # Windows are t-major: window w of piece p holds candidates
        # 1024*p + (w%256) + 256*t, t = 0..3.
        # A-piece trees: level 1 runs as TWO batched DVE instrs (3 pieces
        # each, bf16 2x); level 2 for the first three A-pieces (sss) runs on
        # GPSIMD via max(x,y)=y+relu(x-y), for the last three as ONE batched
        # DVE instr. wm slots are written in a permuted order (batched ops
        # need contiguous slots); the host maps slot->piece via WM_PERM.
        for t in range(NT):
            abuf = ab_pool.tile([128, 6, PIECE], BF16, tag="ab")
            l1b = l1_pool.tile([128, 6, 2, PIECE // 4], BF16, tag="l1")
            wm = wm_pool.tile([128, NPIECE, PIECE // 4], BF16, tag="wm")
            na = 0
            nd = 0
            for p in range(NPIECE):
                Q = PIECE // 4
                ps = psum_pool.tile([128, PIECE], F32, tag="ps")
                for c0 in range(0, PIECE, MMCH):
                    gcol = p * PIECE + c0      # global candidate column
                    g = gcol // GW             # band
                    off = gcol - g * GW
                    nc.tensor.matmul(
                        ps[:, c0:c0 + MMCH],
                        lhsT=lhsT_sb[32 * g:32 * g + KMM,
                                     t * 128:(t + 1) * 128],
                        rhs=rhs_sb[32 * g:32 * g + KMM, off:off + MMCH],
                        start=True, stop=True,
                        tile_position=(32 * g, 0),
                    )
                if KINDS[p] == "A":
                    nc.scalar.copy(abuf[:, na, :], ps[:])
                    na += 1
                    if na in (3, 6):
                        lo = na - 3
                        nc.vector.tensor_tensor(
                            l1b[:, lo:na],
                            abuf[:, lo:na, 0:2 * Q].rearrange(
                                "p n (l w) -> p n l w", l=2),
                            abuf[:, lo:na, 2 * Q:4 * Q].rearrange(
                                "p n (l w) -> p n l w", l=2), ALU.max)
                        if na == 3:   # level 2 on GPSIMD (3-op max identity)
                            for j in range(3):
                                tmp = l1_pool.tile([128, Q], BF16, tag="tmpp")
                                nc.gpsimd.tensor_tensor(
                                    tmp[:], l1b[:, j, 0, :], l1b[:, j, 1, :],
                                    ALU.subtract)
                                nc.gpsimd.tensor_scalar(tmp[:], tmp[:], 0.0,
                                                        None, ALU.max)
                                nc.gpsimd.tensor_tensor(
                                    wm[:, 3 + j, :], l1b[:, j, 1, :], tmp[:],
                                    ALU.add)
                        else:         # level 2 batched on DVE (bf16 2x)
                            nc.vector.tensor_tensor(
                                wm[:, 0:3, :], l1b[:, 3:6, 0, :],
                                l1b[:, 3:6, 1, :], ALU.max)
                else:
                    nc.vector.tensor_reduce(
                        wm[:, 6 + nd, :],
                        ps[:].rearrange("p (t w) -> p w t", t=4),
                        mybir.AxisListType.X, ALU.max)
                    nd += 1
            eng = (nc.sync, nc.scalar)[t % 2]
            eng.dma_start(wm_out[t], wm[:].rearrange("p a b -> p (a b)"))

    nc.compile()
    return nc


def _build_prog2(ts):
    nc = bacc.Bacc("TRN2", target_bir_lowering=False, debug=False,
                   num_devices=NCORES)
    # Deduplicated symmetric pairs: hit(i,j,e1,e2) == hit(j,i,e2,e1), so the
    # host sends each unique face pair once (packed densely, any pair in any
    # slot) and assembles per-row crossing counts from the shared results.
    gA_in = nc.dram_tensor("gA", [128, ts, 18], F32, kind="ExternalInput").ap()
    gB_in = nc.dram_tensor("gB", [128, ts, 18], F32, kind="ExternalInput").ap()
    hr_out = nc.dram_tensor("hr", [128, ts], F32, kind="ExternalOutput").ap()

    with tile.TileContext(nc) as tc, ExitStack() as ctx:
        pool = ctx.enter_context(tc.tile_pool(name="p", bufs=1))

        TS = ts
        NQ = 4
        QW = TS // NQ
        # chunk-by-chunk DMA on alternating queues: chunk q's compute starts
        # as soon as its own slices land (HWDGE serializes descriptors)
        gA = pool.tile([128, TS, 18], F32)
        gB = pool.tile([128, TS, 18], F32)
        for q in range(NQ):
            qsl = slice(q * QW, (q + 1) * QW)
            nc.sync.dma_start(gA[:, qsl, :], gA_in[:, qsl, :])
            nc.scalar.dma_start(gB[:, qsl, :], gB_in[:, qsl, :])

        hwsum = pool.tile([128, TS], F32)

        def mk_views(x0, x1):
            nx = x1 - x0
            SH = [128, nx, 3, 3]
            xsl = slice(x0, x1)

            def uc(c):   # side-A edge dir comp c (varies e1)
                return gA[:, xsl, 9 + c:18:3].unsqueeze(3).broadcast_to(SH)

            def sc(c):   # side-A edge start comp c
                return gA[:, xsl, c:9:3].unsqueeze(3).broadcast_to(SH)

            def vc(c):   # side-B edge dir comp c (varies e2)
                return gB[:, xsl, 9 + c:18:3].unsqueeze(2).broadcast_to(SH)

            def tcp(c):  # side-B edge start comp c
                return gB[:, xsl, c:9:3].unsqueeze(2).broadcast_to(SH)

            return SH, xsl, uc, sc, vc, tcp

        DV = nc.vector.tensor_tensor
        GP = nc.gpsimd.tensor_tensor

        def emit_a1(x0, x1):
            """Stage A1: broadcast products (DVE-only), ordered so the
            GPSIMD consumers (cr1/cr2 then t0/t1) unblock earliest."""
            SH, xsl, uc, sc, vc, tcp = mk_views(x0, x1)
            pfx = f"e{x0}"
            m = [pool.tile(SH, F32, name=f"{pfx}_m{i}") for i in range(6)]
            dif = [pool.tile(SH, F32, name=f"{pfx}_d{i}") for i in range(3)]
            for i in (1, 2, 0):  # cr_i = u_{i+1}*v_{i+2} - u_{i+2}*v_{i+1}
                a, b = (i + 1) % 3, (i + 2) % 3
                DV(m[2 * i][:], uc(a), vc(b), ALU.mult)
                DV(m[2 * i + 1][:], uc(b), vc(a), ALU.mult)
                if i != 0:
                    GP_cr(pfx, SH, m, i)
            for c in (1, 2, 0):
                DV(dif[c][:], tcp(c), sc(c), ALU.subtract)
            return m, dif

        _cr = {}

        def GP_cr(pfx, SH, m, i):
            cr = pool.tile(SH, F32, name=f"{pfx}_cr{i}")
            GP(cr[:], m[2 * i][:], m[2 * i + 1][:], ALU.subtract)
            _cr[(pfx, i)] = cr

        def emit_a2(x0, x1, m, dif):
            """Stage A2: the unit-stride num/den chain (DVE+GPSIMD+ACT)."""
            SH, xsl, *_ = mk_views(x0, x1)
            pfx = f"e{x0}"
            cr0 = pool.tile(SH, F32, name=f"{pfx}_cr0")
            DV(cr0[:], m[0][:], m[1][:], ALU.subtract)
            cr1, cr2 = _cr[(pfx, 1)], _cr[(pfx, 2)]

            num = pool.tile(SH, F32, name=f"{pfx}_num")
            t0 = pool.tile(SH, F32, name=f"{pfx}_t0")
            t1 = pool.tile(SH, F32, name=f"{pfx}_t1")
            GP(t0[:], dif[1][:], cr1[:], ALU.mult)
            GP(t1[:], dif[2][:], cr2[:], ALU.mult)
            DV(num[:], dif[0][:], cr0[:], ALU.mult)
            DV(num[:], num[:], t0[:], ALU.add)
            GP(num[:], num[:], t1[:], ALU.add)

            # den2 = cr0^2 + cr1^2 + cr2^2: squares on ACT
            s0 = pool.tile(SH, F32, name=f"{pfx}_s0")
            s1 = pool.tile(SH, F32, name=f"{pfx}_s1")
            s2 = pool.tile(SH, F32, name=f"{pfx}_s2")
            nc.scalar.activation(s1[:], cr1[:], AFT.Square)
            nc.scalar.activation(s2[:], cr2[:], AFT.Square)
            nc.scalar.activation(s0[:], cr0[:], AFT.Square)
            GP(s0[:], s0[:], s1[:], ALU.add)
            GP(s0[:], s0[:], s2[:], ALU.add)
            # (num/eps)^2 on ACT (scale folded into the square): the hit test
            # num^2 < eps^2*den2 becomes (num/eps)^2 < den2 directly.
            num2 = pool.tile(SH, F32, name=f"{pfx}_n2")
            nc.scalar.activation(num2[:], num[:], AFT.Square,
                                 scale=float(1.0 / EPS))
            return num2, s0

        def emit_b(x0, x1, num2, s0):
            """Stage B: hit test + per-pair 3x3 hit count."""
            nx = x1 - x0
            SH = [128, nx, 3, 3]
            xsl = slice(x0, x1)
            pfx = f"e{x0}"
            hit = pool.tile(SH, F32, name=f"{pfx}_hit")
            DV(hit[:], num2[:], s0[:], ALU.is_lt)    # GPSIMD lacks compares
            nc.vector.tensor_reduce(
                hwsum[:, xsl], hit[:].rearrange("p s a b -> p s (a b)"),
                mybir.AxisListType.X, ALU.add)

        # 3-stage software pipeline: A1(q) | A2(q-1) | B(q-2)
        bounds = [(q * TS // NQ, (q + 1) * TS // NQ) for q in range(NQ)]
        st_a = {}
        st_b = {}
        for q in range(NQ + 2):
            if q < NQ:
                st_a[q] = emit_a1(*bounds[q])
            if 1 <= q <= NQ:
                st_b[q - 1] = emit_a2(*bounds[q - 1], *st_a.pop(q - 1))
            if q >= 2:
                emit_b(*bounds[q - 2], *st_b.pop(q - 2))

        nc.sync.dma_start(hr_out[:], hwsum[:])

    nc.compile()
    return nc


_PROGS = {}


def _get_prog1():
    if "p1" not in _PROGS:
        _PROGS["p1"] = _build_prog1()
    return _PROGS["p1"]


def _get_prog2(ts):
    key = f"p2_{ts}"
    if key not in _PROGS:
        _PROGS[key] = _build_prog2(ts)
        _PROGS["p2_last"] = _PROGS[key]
    return _PROGS[key]


def _get_progs():
    """Compat: (prog1, most recently built prog2)."""
    if "p2_last" not in _PROGS:
        _get_prog2(108)
    return _get_prog1(), _PROGS["p2_last"]


def _host_prep(vertices, faces, probabilities):
    V = np.ascontiguousarray(vertices, dtype=np.float32)
    Fc = np.ascontiguousarray(faces).astype(np.int64)
    P = np.ascontiguousarray(probabilities, dtype=np.float32)
    F = Fc.shape[0]

    pos = V[Fc]                                             # [F,3,3]
    bary = (pos[:, 0] + pos[:, 1] + pos[:, 2]) / np.float32(3.0)
    sq = (bary * bary).sum(-1, dtype=np.float32)

    bf = ml_dtypes.bfloat16
    bh = bary.astype(bf).astype(np.float32)
    bl = (bary - bh).astype(bf).astype(np.float32)
    sqh = sq.astype(bf).astype(np.float32)
    sql = (sq - sqh).astype(bf).astype(np.float32)

    rhs = np.zeros((KMM, FP), np.float32)
    rhs[0:3, :F] = (2.0 * bh).T
    rhs[3:6, :F] = (2.0 * bl).T
    rhs[6:9, :F] = (2.0 * bh).T
    rhs[9:12, :F] = (2.0 * bl).T
    rhs[12, :] = -1.0
    rhs[13, :] = -1.0
    rhs[14, :F] = -sqh
    rhs[15, :F] = -sql
    rhs[14, F:] = -1.0e30
    # band b at partitions [32b, 32b+16) holds candidates [b*GW, (b+1)*GW)
    rhs_bf = rhs.astype(bf)
    rhs_b = np.zeros((128, GW), bf)
    for b in range(NGRP):
        rhs_b[32 * b:32 * b + KMM] = rhs_bf[:, b * GW:(b + 1) * GW]

    lhsT = np.zeros((KMM, FP), np.float32)
    lhsT[0:3, :F] = bh.T
    lhsT[3:6, :F] = bh.T
    lhsT[6:9, :F] = bl.T
    lhsT[9:12, :F] = bl.T
    lhsT[12, :F] = sqh
    lhsT[13, :F] = sql
    lhsT[14, :] = 1.0
    lhsT[15, :] = 1.0
    lhsT_bf = lhsT.astype(bf)
    lhsT_b = np.zeros((128, FP), bf)
    for b in range(NGRP):
        lhsT_b[32 * b:32 * b + KMM] = lhsT_bf

    starts = pos[:, [0, 0, 1], :].reshape(F, 9)
    dirs = (pos[:, [1, 2, 2], :] - pos[:, [0, 0, 1], :]).reshape(F, 9)
    geo = np.zeros((FP, 18), np.float32)
    geo[:F, 0:9] = starts
    geo[:F, 9:18] = dirs

    probs_pad = np.zeros(FP, np.float32)
    probs_pad[:F] = P

    bary_pad = np.zeros((FP, 3), np.float32)
    bary_pad[:F] = bary
    sq_pad = np.zeros(FP, np.float32)
    sq_pad[:F] = sq

    in1 = []
    for c in range(NCORES):
        lo, hi = c * NR, (c + 1) * NR
        in1.append({
            "lhsT": np.ascontiguousarray(lhsT_b[:, lo:hi]),
            "rhs": rhs_b,
        })
    aux = dict(F=F, geo=geo, probs_pad=probs_pad,
               bary=bary, sq=sq, bh=bh, bl=bl, sqh=sqh, sql=sql,
               bary_pad=bary_pad, sq_pad=sq_pad)
    return in1, aux


def _exact_rows_negd2(rows, aux):
    """Replicate the device -d2 rows in f32 (bf16-split products, f32 sums)."""
    bh, bl, sqh, sql = aux["bh"], aux["bl"], aux["sqh"], aux["sql"]
    F = aux["F"]
    rows = np.asarray(rows)
    live = rows < F                     # pad query rows have all-zero terms
    rc = np.where(live, rows, 0)
    S = len(rows)
    acc = np.zeros((S, FP), np.float32)
    for qp, cp in ((bh, bh), (bl, bh), (bh, bl), (bl, bl)):
        acc[:, :F] += (2 * qp[rc] * live[:, None]) @ cp.T
    acc[:, :F] -= ((sqh[rc] + sql[rc]) * live)[:, None]
    acc[:, :F] -= (sqh + sql)[None, :F]
    acc[:, F:] = -1.0e30
    return acc


def _host_merge(res1, aux):
    """Window-max merge: resolve the top-MWIN windows per row exactly;
    value-margin fallback to a full exact recompute. Returns nbr [FP, 16]."""
    F = aux["F"]
    wm = np.empty((FP, NWIN), np.float32)
    for c in range(NCORES):
        wm[c * NR:(c + 1) * NR] = np.asarray(
            res1.results[c]["wm"]).astype(np.float32).reshape(NR, NWIN)

    part = np.argpartition(-wm, MWIN, axis=1)
    topw = part[:, :MWIN]                                   # [FP, MWIN]
    w25 = np.take_along_axis(
        wm, part[:, MWIN:MWIN + 1], axis=1)[:, 0]           # (MWIN+1)-th value
    # rank within the partition isn't sorted; w25 must be max of the rest
    rest = np.take_along_axis(wm, part[:, MWIN:], axis=1)
    w25 = rest.max(axis=1)

    # window W (0..2559) in wm slot k=W//256 maps to piece WM_PERM[k] and
    # holds candidates 1024*piece + (W%256) + 256*t, t=0..3 (t-major)
    k = WM_PERM[topw // 256]
    w = topw % 256
    cand = (1024 * k + w)[:, :, None] + \
        (256 * np.arange(4, dtype=np.int64))[None, None, :]
    cand = cand.reshape(FP, MWIN * 4)                       # [FP, 128]

    bary_pad, sq_pad = aux["bary_pad"], aux["sq_pad"]
    dot = np.einsum("rd,rmd->rm", bary_pad, bary_pad[cand], optimize=True)
    v = 2.0 * dot - sq_pad[:, None] - sq_pad[cand]
    v = v.astype(np.float32)
    v[cand >= F] = -1.0e30

    prt = np.argpartition(-v, KNN, axis=1)[:, :KNN]
    pv = np.take_along_axis(v, prt, axis=1)
    pc = np.take_along_axis(cand, prt, axis=1)
    order = np.lexsort((pc, -pv), axis=1)
    nbr = np.take_along_axis(pc, order, axis=1)             # [FP, 16]
    nv = np.take_along_axis(pv, order, axis=1)
    v16 = nv[:, KNN - 1]

    # fallback: the (MWIN+1)-th window max (plus bf16 + recompute margin)
    # could hide an unresolved true top-16 member.
    margin = np.abs(w25) * (2.0 ** -7) + 1e-5
    suspect = np.nonzero((w25 + margin >= v16) & (np.arange(FP) < F))[0]
    if suspect.size:
        negd2 = _exact_rows_negd2(suspect, aux)
        sp = np.argpartition(-negd2, KNN, axis=1)[:, :KNN]
        spv = np.take_along_axis(negd2, sp, axis=1)
        o = np.lexsort((sp, -spv), axis=1)
        nbr[suspect] = np.take_along_axis(sp, o, axis=1)
    return nbr


def _run(vertices, faces, probabilities, trace=False, **kw):
    p1 = _get_prog1()
    in1, aux = _host_prep(vertices, faces, probabilities)
    res1 = run_bass_kernel_spmd(p1, in1, list(range(NCORES)), trace=trace, **kw)
    nbr = _host_merge(res1, aux)                            # [FP, 16]
    F = aux["F"]
    geo = aux["geo"]

    # deduplicate symmetric pairs: hit(i,j) == hit(j,i) summed over the 3x3
    # edge grid, so each unique face pair is tested once on device and the
    # per-row crossing counts are assembled from the shared results.
    i = np.repeat(np.arange(FP, dtype=np.int64), KNN)
    j = nbr.reshape(-1).astype(np.int64)
    valid = (j != i) & (i < F)
    iv, jv = i[valid], j[valid]
    key = np.minimum(iv, jv) * FP + np.maximum(iv, jv)
    uniq, inv = np.unique(key, return_inverse=True)
    U = uniq.size
    percore = -(-U // NCORES)
    ts = max(8, (-(-percore // 128) + 3) // 4 * 4)          # slots, mult of 4
    cap = NCORES * 128 * ts
    ua = uniq // FP
    ub = uniq % FP
    gAf = np.zeros((cap, 18), np.float32)
    gBf = np.zeros((cap, 18), np.float32)
    gAf[:U] = geo[ua]
    gBf[:U] = geo[ub]

    p2 = _get_prog2(ts)
    in2 = []
    for c in range(NCORES):
        blk = slice(c * 128 * ts, (c + 1) * 128 * ts)
        in2.append({
            "gA": np.ascontiguousarray(
                gAf[blk].reshape(ts, 128, 18).transpose(1, 0, 2)),
            "gB": np.ascontiguousarray(
                gBf[blk].reshape(ts, 128, 18).transpose(1, 0, 2)),
        })
    res2 = run_bass_kernel_spmd(p2, in2, list(range(NCORES)), trace=trace, **kw)

    hit_u = np.concatenate([
        np.asarray(res2.results[c]["hr"]).reshape(128, ts).T.reshape(-1)
        for c in range(NCORES)])[:U]
    total = np.dot(aux["probs_pad"][iv].astype(np.float64),
                   hit_u[inv].astype(np.float64))
    loss = np.float32(total / F)
    return loss, res1, res2, nbr


def run_device(vertices, faces, probabilities, trace=False, **kw):
    loss, res1, res2, _ = _run(vertices, faces, probabilities, trace=trace, **kw)
    return loss, (res1, res2)


def kernel(vertices, faces, probabilities):
    loss, *_ = _run(vertices, faces, probabilities)
    return np.array(loss, dtype=np.float32)


# revision 77
# speedup vs baseline: 1.0035x; 1.0035x over previous
"""EdgeCrossingsLoss Trainium2 kernel (8-core SPMD, data-parallel over query faces).

Two device launches (no usable on-device gather in this runtime; the host does
the small index-merge + geometry gather between the launches):

prog1 (per core, 1280 query rows = 10 tiles of 128):
  PE:  -d2[q, c] for all 10240 candidates via a K=16 bf16 hi/lo-split matmul
       (bf16 products are exact, accumulated in f32 PSUM). rhs sits in four
       16-partition bands at base partitions 0/32/64/96. PSUM pieces are
       [128, 1024] f32 (2 banks), a 4-deep ring.
  Reduction: instead of two full f32 DVE scans (max8 + max_index, the old
       bottleneck), window-4 maxima of -d2 are computed in bf16 and shipped
       whole. TRN2 limits: GPSIMD cannot touch PSUM (and lacks max/compare
       ops in this runtime); a TensorTensor reads at most one PSUM operand.
       So 4 pieces/tile drain via DVE tensor_reduce (single strided PSUM
       input, reduce AND window in one op), 6 drain via ACT copy to bf16
       SBUF followed by a 2-level contiguous-half pair-max tree on DVE (bf16
       2x mode; one level-2 per tile on GPSIMD via max(x,y)=y+relu(x-y)).
       The [128, 2560] window-max tile is DMA'd out; no on-device top-k.

host: picks the top-32 windows per row from the 2560 bf16 window maxima
      (window W holds candidates 1024*(W//256) + W%256 + 256*t, t=0..3),
      resolves the 32*4 member candidates exactly (f32), takes the exact
      top-16 with the jax tie-break. Rows where the 33rd-best window max
      could hide a true top-16 member (value margin covering bf16 rounding)
      are recomputed exactly (vectorized, ~a few % of rows). Gathers the 16
      neighbor faces' edge geometry; folds probabilities and the
      self-neighbor mask into per-(row, slot) weights.

prog2 (per core): 3x3 line-line crossing tests over DEDUPLICATED face pairs.
      The 3x3-summed hit count is symmetric (hit(i,j) == hit(j,i)), so the
      host sends each unique pair once, packed densely across cores (~65% of
      the raw 10000x16 pair slots), and assembles per-row crossing counts
      from the shared results. Engine-split across four software-pipelined
      slot-chunks: DVE runs the broadcast-AP ops, the compare and the
      reduction + part of the unit-stride chain, GPSIMD runs adds/subs/mults,
      ACT squares cr^2 and (num/eps)^2 (eps folded into the square's scale).
      hit = (num/eps)^2 < |cross|^2 (den=0 / NaN / zero-padded slots fall out
      correctly as no-hit).

Host computes loss = dot(p_i over valid (i,j) slots, hit_unique[pair]) / F.
"""
import os
import numpy as np
import ml_dtypes
from contextlib import ExitStack

import concourse.bass as bass
import concourse.tile as tile
import concourse.bacc as bacc
from concourse import mybir
from concourse.bass_utils import run_bass_kernel_spmd

F32 = mybir.dt.float32
BF16 = mybir.dt.bfloat16
U16 = mybir.dt.uint16

NCORES = 8
KNN = 16
EPS = 1e-5
FP = 10240            # padded candidate count
NR = FP // NCORES     # 1280 rows per core
NT = NR // 128        # 10 tiles of 128 rows
KMM = 16              # matmul contraction rows (bf16 hi/lo split)
NGRP = 4              # rhs partition bands (at partitions 0/32/64/96)
GW = FP // NGRP       # 2560 candidates per band
PIECE = 1024          # PSUM piece width (f32, exactly 2 banks -> 4-deep ring)
NPIECE = FP // PIECE  # 10 pieces per tile
MMCH = 512            # matmul N per instruction (one PSUM bank)
NWIN = FP // 4        # 2560 window-4 maxima per row
MWIN = 32             # host resolves top-32 windows per row
KINDS = "DADAADADAA"  # prog1 piece drain engines: A=ACT copy, D=DVE reduce
# wm slot -> piece map (batched tree instrs need contiguous wm slots):
# slots 0-2 = DVE level-2 batch (A-pieces 6,8,9), 3-5 = GPSIMD level-2
# (A-pieces 1,3,4), 6-9 = DVE reduces (D-pieces 0,2,5,7)
WM_PERM = np.array([6, 8, 9, 1, 3, 4, 0, 2, 5, 7], dtype=np.int64)

ALU = mybir.AluOpType
AFT = mybir.ActivationFunctionType


def _build_prog1():
    nc = bacc.Bacc("TRN2", target_bir_lowering=False, debug=False,
                   num_devices=NCORES)
    # band b occupies partitions [32b, 32b+16); lhsT replicated into each band
    lhsT_in = nc.dram_tensor("lhsT", [128, NR], BF16, kind="ExternalInput").ap()
    rhs_in = nc.dram_tensor("rhs", [128, GW], BF16, kind="ExternalInput").ap()
    wm_out = nc.dram_tensor("wm", [NT, 128, NWIN], BF16,
                            kind="ExternalOutput").ap()

    with tile.TileContext(nc) as tc, ExitStack() as ctx:
        const_pool = ctx.enter_context(tc.tile_pool(name="const", bufs=1))
        psum_pool = ctx.enter_context(tc.tile_pool(name="psum", bufs=4,
                                                   space="PSUM"))
        ab_pool = ctx.enter_context(tc.tile_pool(name="ab", bufs=3))
        l1_pool = ctx.enter_context(tc.tile_pool(name="l1", bufs=6))
        wm_pool = ctx.enter_context(tc.tile_pool(name="wmp", bufs=3))

        # Each DMA chunk gets its OWN tile: a single multi-writer tile makes
        # readers wait for ALL its DMAs (coarse deps), serializing the head.
        # lhsT: tile 0's slice separate; rhs: five 512-col tiles that align
        # exactly with the matmul chunks (2560 = 5*512).
        lhsT0_sb = const_pool.tile([128, 128], BF16, name="lhsT0")
        nc.sync.dma_start(lhsT0_sb[:], lhsT_in[:, 0:128])
        rhs_sb = [const_pool.tile([128, MMCH], BF16, name=f"rhs{j}")
                  for j in range(5)]
        for j in range(5):
            eng = (nc.scalar, nc.sync)[j % 2]
            eng.dma_start(rhs_sb[j][:], rhs_in[:, j * MMCH:(j + 1) * MMCH])
        # lhsT for tiles 1..9 is only needed ~7us in: issue it last
        lhsT1_sb = const_pool.tile([128, NR - 128], BF16, name="lhsT1")
        nc.sync.dma_start(lhsT1_sb[:], lhsT_in[:, 128:])

        def lhsT_ap(g, t):
            if t == 0:
                return lhsT0_sb[32 * g:32 * g + KMM, :]
            return lhsT1_sb[32 * g:32 * g + KMM, (t - 1) * 128:t * 128]

        def rhs_ap(g, off):
            return rhs_sb[off // MMCH][32 * g:32 * g + KMM, :]

        # PSUM-read rules on TRN2: GPSIMD may not touch PSUM at all, and a
        # TensorTensor may read at most ONE operand from PSUM. GPSIMD also
        # lacks max/min/compare ops in this runtime (add/sub/mult and
        # tensor_scalar max only). So: DVE tensor_reduce (single PSUM input,
        # strided [128, 256, 4] view) drains AND window-4-maxes 4 pieces per
        # tile in one op each; ACT copies the other 6 pieces to bf16 SBUF
        # where the 2-level contiguous-half pair-max tree runs on DVE (bf16
        # 2x) or on GPSIMD via the 3-op identity max(x,y) = y + relu(x-y).
        # Windows are t-major: window w of piece p holds candidates
        # 1024*p + (w%256) + 256*t, t = 0..3.
        # A-piece trees: level 1 runs as TWO batched DVE instrs (3 pieces
        # each, bf16 2x); level 2 for the first three A-pieces (sss) runs on
        # GPSIMD via max(x,y)=y+relu(x-y), for the last three as ONE batched
        # DVE instr. wm slots are written in a permuted order (batched ops
        # need contiguous slots); the host maps slot->piece via WM_PERM.
        for t in range(NT):
            abuf = ab_pool.tile([128, 6, PIECE], BF16, tag="ab")
            l1b = l1_pool.tile([128, 6, 2, PIECE // 4], BF16, tag="l1")
            wm = wm_pool.tile([128, NPIECE, PIECE // 4], BF16, tag="wm")
            na = 0
            nd = 0
            for p in range(NPIECE):
                Q = PIECE // 4
                ps = psum_pool.tile([128, PIECE], F32, tag="ps")
                for c0 in range(0, PIECE, MMCH):
                    gcol = p * PIECE + c0      # global candidate column
                    g = gcol // GW             # band
                    off = gcol - g * GW
                    nc.tensor.matmul(
                        ps[:, c0:c0 + MMCH],
                        lhsT=lhsT_ap(g, t),
                        rhs=rhs_ap(g, off),
                        start=True, stop=True,
                        tile_position=(32 * g, 0),
                    )
                if KINDS[p] == "A":
                    nc.scalar.copy(abuf[:, na, :], ps[:])
                    na += 1
                    if na in (3, 6):
                        lo = na - 3
                        nc.vector.tensor_tensor(
                            l1b[:, lo:na],
                            abuf[:, lo:na, 0:2 * Q].rearrange(
                                "p n (l w) -> p n l w", l=2),
                            abuf[:, lo:na, 2 * Q:4 * Q].rearrange(
                                "p n (l w) -> p n l w", l=2), ALU.max)
                        if na == 3:   # level 2 on GPSIMD (3-op max identity)
                            for j in range(3):
                                tmp = l1_pool.tile([128, Q], BF16, tag="tmpp")
                                nc.gpsimd.tensor_tensor(
                                    tmp[:], l1b[:, j, 0, :], l1b[:, j, 1, :],
                                    ALU.subtract)
                                nc.gpsimd.tensor_scalar(tmp[:], tmp[:], 0.0,
                                                        None, ALU.max)
                                nc.gpsimd.tensor_tensor(
                                    wm[:, 3 + j, :], l1b[:, j, 1, :], tmp[:],
                                    ALU.add)
                        else:         # level 2 batched on DVE (bf16 2x)
                            nc.vector.tensor_tensor(
                                wm[:, 0:3, :], l1b[:, 3:6, 0, :],
                                l1b[:, 3:6, 1, :], ALU.max)
                else:
                    nc.vector.tensor_reduce(
                        wm[:, 6 + nd, :],
                        ps[:].rearrange("p (t w) -> p w t", t=4),
                        mybir.AxisListType.X, ALU.max)
                    nd += 1
            eng = (nc.sync, nc.scalar)[t % 2]
            eng.dma_start(wm_out[t], wm[:].rearrange("p a b -> p (a b)"))

    nc.compile()
    return nc


def _build_prog2(ts):
    nc = bacc.Bacc("TRN2", target_bir_lowering=False, debug=False,
                   num_devices=NCORES)
    # Deduplicated symmetric pairs: hit(i,j,e1,e2) == hit(j,i,e2,e1), so the
    # host sends each unique face pair once (packed densely, any pair in any
    # slot) and assembles per-row crossing counts from the shared results.
    gA_in = nc.dram_tensor("gA", [128, ts, 18], F32, kind="ExternalInput").ap()
    gB_in = nc.dram_tensor("gB", [128, ts, 18], F32, kind="ExternalInput").ap()
    hr_out = nc.dram_tensor("hr", [128, ts], F32, kind="ExternalOutput").ap()

    with tile.TileContext(nc) as tc, ExitStack() as ctx:
        pool = ctx.enter_context(tc.tile_pool(name="p", bufs=1))

        TS = ts
        NQ = 4
        QW = TS // NQ
        # chunk-by-chunk DMA on alternating queues: chunk q's compute starts
        # as soon as its own slices land (HWDGE serializes descriptors)
        gA = pool.tile([128, TS, 18], F32)
        gB = pool.tile([128, TS, 18], F32)
        for q in range(NQ):
            qsl = slice(q * QW, (q + 1) * QW)
            nc.sync.dma_start(gA[:, qsl, :], gA_in[:, qsl, :])
            nc.scalar.dma_start(gB[:, qsl, :], gB_in[:, qsl, :])

        hwsum = pool.tile([128, TS], F32)

        def mk_views(x0, x1):
            nx = x1 - x0
            SH = [128, nx, 3, 3]
            xsl = slice(x0, x1)

            def uc(c):   # side-A edge dir comp c (varies e1)
                return gA[:, xsl, 9 + c:18:3].unsqueeze(3).broadcast_to(SH)

            def sc(c):   # side-A edge start comp c
                return gA[:, xsl, c:9:3].unsqueeze(3).broadcast_to(SH)

            def vc(c):   # side-B edge dir comp c (varies e2)
                return gB[:, xsl, 9 + c:18:3].unsqueeze(2).broadcast_to(SH)

            def tcp(c):  # side-B edge start comp c
                return gB[:, xsl, c:9:3].unsqueeze(2).broadcast_to(SH)

            return SH, xsl, uc, sc, vc, tcp

        DV = nc.vector.tensor_tensor
        GP = nc.gpsimd.tensor_tensor

        def emit_a1(x0, x1):
            """Stage A1: broadcast products (DVE-only), ordered so the
            GPSIMD consumers (cr1/cr2 then t0/t1) unblock earliest."""
            SH, xsl, uc, sc, vc, tcp = mk_views(x0, x1)
            pfx = f"e{x0}"
            m = [pool.tile(SH, F32, name=f"{pfx}_m{i}") for i in range(6)]
            dif = [pool.tile(SH, F32, name=f"{pfx}_d{i}") for i in range(3)]
            for i in (1, 2, 0):  # cr_i = u_{i+1}*v_{i+2} - u_{i+2}*v_{i+1}
                a, b = (i + 1) % 3, (i + 2) % 3
                DV(m[2 * i][:], uc(a), vc(b), ALU.mult)
                DV(m[2 * i + 1][:], uc(b), vc(a), ALU.mult)
                if i != 0:
                    GP_cr(pfx, SH, m, i)
            for c in (1, 2, 0):
                DV(dif[c][:], tcp(c), sc(c), ALU.subtract)
            return m, dif

        _cr = {}

        def GP_cr(pfx, SH, m, i):
            cr = pool.tile(SH, F32, name=f"{pfx}_cr{i}")
            GP(cr[:], m[2 * i][:], m[2 * i + 1][:], ALU.subtract)
            _cr[(pfx, i)] = cr

        def emit_a2(x0, x1, m, dif):
            """Stage A2: the unit-stride num/den chain (DVE+GPSIMD+ACT)."""
            SH, xsl, *_ = mk_views(x0, x1)
            pfx = f"e{x0}"
            cr0 = pool.tile(SH, F32, name=f"{pfx}_cr0")
            DV(cr0[:], m[0][:], m[1][:], ALU.subtract)
            cr1, cr2 = _cr[(pfx, 1)], _cr[(pfx, 2)]

            num = pool.tile(SH, F32, name=f"{pfx}_num")
            t0 = pool.tile(SH, F32, name=f"{pfx}_t0")
            t1 = pool.tile(SH, F32, name=f"{pfx}_t1")
            GP(t0[:], dif[1][:], cr1[:], ALU.mult)
            GP(t1[:], dif[2][:], cr2[:], ALU.mult)
            DV(num[:], dif[0][:], cr0[:], ALU.mult)
            DV(num[:], num[:], t0[:], ALU.add)
            GP(num[:], num[:], t1[:], ALU.add)

            # den2 = cr0^2 + cr1^2 + cr2^2: squares on ACT
            s0 = pool.tile(SH, F32, name=f"{pfx}_s0")
            s1 = pool.tile(SH, F32, name=f"{pfx}_s1")
            s2 = pool.tile(SH, F32, name=f"{pfx}_s2")
            nc.scalar.activation(s1[:], cr1[:], AFT.Square)
            nc.scalar.activation(s2[:], cr2[:], AFT.Square)
            nc.scalar.activation(s0[:], cr0[:], AFT.Square)
            GP(s0[:], s0[:], s1[:], ALU.add)
            GP(s0[:], s0[:], s2[:], ALU.add)
            # (num/eps)^2 on ACT (scale folded into the square): the hit test
            # num^2 < eps^2*den2 becomes (num/eps)^2 < den2 directly.
            num2 = pool.tile(SH, F32, name=f"{pfx}_n2")
            nc.scalar.activation(num2[:], num[:], AFT.Square,
                                 scale=float(1.0 / EPS))
            return num2, s0

        def emit_b(x0, x1, num2, s0):
            """Stage B: hit test + per-pair 3x3 hit count."""
            nx = x1 - x0
            SH = [128, nx, 3, 3]
            xsl = slice(x0, x1)
            pfx = f"e{x0}"
            hit = pool.tile(SH, F32, name=f"{pfx}_hit")
            DV(hit[:], num2[:], s0[:], ALU.is_lt)    # GPSIMD lacks compares
            nc.vector.tensor_reduce(
                hwsum[:, xsl], hit[:].rearrange("p s a b -> p s (a b)"),
                mybir.AxisListType.X, ALU.add)

        # 3-stage software pipeline: A1(q) | A2(q-1) | B(q-2)
        bounds = [(q * TS // NQ, (q + 1) * TS // NQ) for q in range(NQ)]
        st_a = {}
        st_b = {}
        for q in range(NQ + 2):
            if q < NQ:
                st_a[q] = emit_a1(*bounds[q])
            if 1 <= q <= NQ:
                st_b[q - 1] = emit_a2(*bounds[q - 1], *st_a.pop(q - 1))
            if q >= 2:
                emit_b(*bounds[q - 2], *st_b.pop(q - 2))

        nc.sync.dma_start(hr_out[:], hwsum[:])

    nc.compile()
    return nc


_PROGS = {}


def _get_prog1():
    if "p1" not in _PROGS:
        _PROGS["p1"] = _build_prog1()
    return _PROGS["p1"]


def _get_prog2(ts):
    key = f"p2_{ts}"
    if key not in _PROGS:
        _PROGS[key] = _build_prog2(ts)
        _PROGS["p2_last"] = _PROGS[key]
    return _PROGS[key]


def _get_progs():
    """Compat: (prog1, most recently built prog2)."""
    if "p2_last" not in _PROGS:
        _get_prog2(108)
    return _get_prog1(), _PROGS["p2_last"]


def _host_prep(vertices, faces, probabilities):
    V = np.ascontiguousarray(vertices, dtype=np.float32)
    Fc = np.ascontiguousarray(faces).astype(np.int64)
    P = np.ascontiguousarray(probabilities, dtype=np.float32)
    F = Fc.shape[0]

    pos = V[Fc]                                             # [F,3,3]
    bary = (pos[:, 0] + pos[:, 1] + pos[:, 2]) / np.float32(3.0)
    sq = (bary * bary).sum(-1, dtype=np.float32)

    bf = ml_dtypes.bfloat16
    bh = bary.astype(bf).astype(np.float32)
    bl = (bary - bh).astype(bf).astype(np.float32)
    sqh = sq.astype(bf).astype(np.float32)
    sql = (sq - sqh).astype(bf).astype(np.float32)

    rhs = np.zeros((KMM, FP), np.float32)
    rhs[0:3, :F] = (2.0 * bh).T
    rhs[3:6, :F] = (2.0 * bl).T
    rhs[6:9, :F] = (2.0 * bh).T
    rhs[9:12, :F] = (2.0 * bl).T
    rhs[12, :] = -1.0
    rhs[13, :] = -1.0
    rhs[14, :F] = -sqh
    rhs[15, :F] = -sql
    rhs[14, F:] = -1.0e30
    # band b at partitions [32b, 32b+16) holds candidates [b*GW, (b+1)*GW)
    rhs_bf = rhs.astype(bf)
    rhs_b = np.zeros((128, GW), bf)
    for b in range(NGRP):
        rhs_b[32 * b:32 * b + KMM] = rhs_bf[:, b * GW:(b + 1) * GW]

    lhsT = np.zeros((KMM, FP), np.float32)
    lhsT[0:3, :F] = bh.T
    lhsT[3:6, :F] = bh.T
    lhsT[6:9, :F] = bl.T
    lhsT[9:12, :F] = bl.T
    lhsT[12, :F] = sqh
    lhsT[13, :F] = sql
    lhsT[14, :] = 1.0
    lhsT[15, :] = 1.0
    lhsT_bf = lhsT.astype(bf)
    lhsT_b = np.zeros((128, FP), bf)
    for b in range(NGRP):
        lhsT_b[32 * b:32 * b + KMM] = lhsT_bf

    starts = pos[:, [0, 0, 1], :].reshape(F, 9)
    dirs = (pos[:, [1, 2, 2], :] - pos[:, [0, 0, 1], :]).reshape(F, 9)
    geo = np.zeros((FP, 18), np.float32)
    geo[:F, 0:9] = starts
    geo[:F, 9:18] = dirs

    probs_pad = np.zeros(FP, np.float32)
    probs_pad[:F] = P

    bary_pad = np.zeros((FP, 3), np.float32)
    bary_pad[:F] = bary
    sq_pad = np.zeros(FP, np.float32)
    sq_pad[:F] = sq

    in1 = []
    for c in range(NCORES):
        lo, hi = c * NR, (c + 1) * NR
        in1.append({
            "lhsT": np.ascontiguousarray(lhsT_b[:, lo:hi]),
            "rhs": rhs_b,
        })
    aux = dict(F=F, geo=geo, probs_pad=probs_pad,
               bary=bary, sq=sq, bh=bh, bl=bl, sqh=sqh, sql=sql,
               bary_pad=bary_pad, sq_pad=sq_pad)
    return in1, aux


def _exact_rows_negd2(rows, aux):
    """Replicate the device -d2 rows in f32 (bf16-split products, f32 sums)."""
    bh, bl, sqh, sql = aux["bh"], aux["bl"], aux["sqh"], aux["sql"]
    F = aux["F"]
    rows = np.asarray(rows)
    live = rows < F                     # pad query rows have all-zero terms
    rc = np.where(live, rows, 0)
    S = len(rows)
    acc = np.zeros((S, FP), np.float32)
    for qp, cp in ((bh, bh), (bl, bh), (bh, bl), (bl, bl)):
        acc[:, :F] += (2 * qp[rc] * live[:, None]) @ cp.T
    acc[:, :F] -= ((sqh[rc] + sql[rc]) * live)[:, None]
    acc[:, :F] -= (sqh + sql)[None, :F]
    acc[:, F:] = -1.0e30
    return acc


def _host_merge(res1, aux):
    """Window-max merge: resolve the top-MWIN windows per row exactly;
    value-margin fallback to a full exact recompute. Returns nbr [FP, 16]."""
    F = aux["F"]
    wm = np.empty((FP, NWIN), np.float32)
    for c in range(NCORES):
        wm[c * NR:(c + 1) * NR] = np.asarray(
            res1.results[c]["wm"]).astype(np.float32).reshape(NR, NWIN)

    part = np.argpartition(-wm, MWIN, axis=1)
    topw = part[:, :MWIN]                                   # [FP, MWIN]
    w25 = np.take_along_axis(
        wm, part[:, MWIN:MWIN + 1], axis=1)[:, 0]           # (MWIN+1)-th value
    # rank within the partition isn't sorted; w25 must be max of the rest
    rest = np.take_along_axis(wm, part[:, MWIN:], axis=1)
    w25 = rest.max(axis=1)

    # window W (0..2559) in wm slot k=W//256 maps to piece WM_PERM[k] and
    # holds candidates 1024*piece + (W%256) + 256*t, t=0..3 (t-major)
    k = WM_PERM[topw // 256]
    w = topw % 256
    cand = (1024 * k + w)[:, :, None] + \
        (256 * np.arange(4, dtype=np.int64))[None, None, :]
    cand = cand.reshape(FP, MWIN * 4)                       # [FP, 128]

    bary_pad, sq_pad = aux["bary_pad"], aux["sq_pad"]
    dot = np.einsum("rd,rmd->rm", bary_pad, bary_pad[cand], optimize=True)
    v = 2.0 * dot - sq_pad[:, None] - sq_pad[cand]
    v = v.astype(np.float32)
    v[cand >= F] = -1.0e30

    prt = np.argpartition(-v, KNN, axis=1)[:, :KNN]
    pv = np.take_along_axis(v, prt, axis=1)
    pc = np.take_along_axis(cand, prt, axis=1)
    order = np.lexsort((pc, -pv), axis=1)
    nbr = np.take_along_axis(pc, order, axis=1)             # [FP, 16]
    nv = np.take_along_axis(pv, order, axis=1)
    v16 = nv[:, KNN - 1]

    # fallback: the (MWIN+1)-th window max (plus bf16 + recompute margin)
    # could hide an unresolved true top-16 member.
    margin = np.abs(w25) * (2.0 ** -7) + 1e-5
    suspect = np.nonzero((w25 + margin >= v16) & (np.arange(FP) < F))[0]
    if suspect.size:
        negd2 = _exact_rows_negd2(suspect, aux)
        sp = np.argpartition(-negd2, KNN, axis=1)[:, :KNN]
        spv = np.take_along_axis(negd2, sp, axis=1)
        o = np.lexsort((sp, -spv), axis=1)
        nbr[suspect] = np.take_along_axis(sp, o, axis=1)
    return nbr


def _run(vertices, faces, probabilities, trace=False, **kw):
    p1 = _get_prog1()
    in1, aux = _host_prep(vertices, faces, probabilities)
    res1 = run_bass_kernel_spmd(p1, in1, list(range(NCORES)), trace=trace, **kw)
    nbr = _host_merge(res1, aux)                            # [FP, 16]
    F = aux["F"]
    geo = aux["geo"]

    # deduplicate symmetric pairs: hit(i,j) == hit(j,i) summed over the 3x3
    # edge grid, so each unique face pair is tested once on device and the
    # per-row crossing counts are assembled from the shared results.
    i = np.repeat(np.arange(FP, dtype=np.int64), KNN)
    j = nbr.reshape(-1).astype(np.int64)
    valid = (j != i) & (i < F)
    iv, jv = i[valid], j[valid]
    key = np.minimum(iv, jv) * FP + np.maximum(iv, jv)
    uniq, inv = np.unique(key, return_inverse=True)
    U = uniq.size
    percore = -(-U // NCORES)
    ts = max(8, (-(-percore // 128) + 3) // 4 * 4)          # slots, mult of 4
    cap = NCORES * 128 * ts
    ua = uniq // FP
    ub = uniq % FP
    gAf = np.zeros((cap, 18), np.float32)
    gBf = np.zeros((cap, 18), np.float32)
    gAf[:U] = geo[ua]
    gBf[:U] = geo[ub]

    p2 = _get_prog2(ts)
    in2 = []
    for c in range(NCORES):
        blk = slice(c * 128 * ts, (c + 1) * 128 * ts)
        in2.append({
            "gA": np.ascontiguousarray(
                gAf[blk].reshape(ts, 128, 18).transpose(1, 0, 2)),
            "gB": np.ascontiguousarray(
                gBf[blk].reshape(ts, 128, 18).transpose(1, 0, 2)),
        })
    res2 = run_bass_kernel_spmd(p2, in2, list(range(NCORES)), trace=trace, **kw)

    hit_u = np.concatenate([
        np.asarray(res2.results[c]["hr"]).reshape(128, ts).T.reshape(-1)
        for c in range(NCORES)])[:U]
    total = np.dot(aux["probs_pad"][iv].astype(np.float64),
                   hit_u[inv].astype(np.float64))
    loss = np.float32(total / F)
    return loss, res1, res2, nbr


def run_device(vertices, faces, probabilities, trace=False, **kw):
    loss, res1, res2, _ = _run(vertices, faces, probabilities, trace=trace, **kw)
    return loss, (res1, res2)


def kernel(vertices, faces, probabilities):
    loss, *_ = _run(vertices, faces, probabilities)
    return np.array(loss, dtype=np.float32)
